# revision 38
# baseline (speedup 1.0000x reference)
"""DAG-LSTM Trainium2 kernel.

Problem: 2-layer LSTM scanned over a 48-node DAG, batch 1024, hidden 256.
Sharding: pure data parallelism -- batch split 8 x 128 across NeuronCores,
weights replicated, no cross-device traffic.

Key optimizations over the naive scan:
1. Dead-code elimination: the reference returns only the top-layer hidden
   state of the LAST DAG node, so only the ancestor cone of (node 47,
   layer 1) is computed -- 20 of 96 (node, layer) units for this graph.
   The cone is scheduled into ASAP stages; same-stage same-layer units
   form groups of <= 2 whose gates are batched into one matmul stream.
2. Transposed layout (feature-on-partition): states h [2x128 part-chunks,
   B=128 free] f16, c fp32 (f16 c costs ~10x the rel err on HW).  All
   LSTM matmuls run without on-chip transposes; layer-0 x operands are
   read straight out of the host-transposed dags tile.
3. Per-chunk single-bank PSUM accumulation groups (start=True) with the
   bias applied via the activation unit's per-partition bias AP.  This is
   race-free by construction -- cross-engine PSUM preloading (DVE/scalar
   writes + start=False matmul accumulation) was measured to race with
   the matmul stream and corrupt a batch suffix nondeterministically.
4. Chunk order f,g,i,o so sigmoid(f) -> c_new -> tanh(c) -> h, the
   cross-stage critical chain, starts as early as possible; per-group
   merged tanh(c) / c-update / h-product (one DVE op each).
5. Latency-ordered split DMAs across both queues + PE warm-up matmuls
   during the input transfers.
"""

import sys
import numpy as np

sys.path.insert(0, "/opt/trn_rl_repo")

B, N, IN, H, L, P = 1024, 48, 256, 256, 2, 2
NCORES = 8
BL = B // NCORES          # 128 batch per core
KC = 2                    # K chunks (256 = 2*128)
GROUP_MAX = 2             # units per (stage, layer) group

_CACHE = {}


def _unit_deps(pred, i, l):
    d = [(int(v) - 1, l) for v in pred[i] if v > 0]
    if l == 1:
        d.append((i, 0))
    return d


def _build_schedule(pred):
    """Ancestor cone of (N-1, 1) scheduled into ASAP stages; same-stage
    same-layer units grouped up to GROUP_MAX.  Returns list of
    (layer, [nodes]) in dependency order."""
    cone = set()
    stack = [(N - 1, 1)]
    while stack:
        u = stack.pop()
        if u in cone:
            continue
        cone.add(u)
        stack.extend(_unit_deps(pred, *u))
    stage = {}
    for u in sorted(cone):
        ds = [d for d in _unit_deps(pred, *u) if d in cone]
        stage[u] = max([stage[d] for d in ds], default=-1) + 1
    nstages = max(stage.values()) + 1
    groups = []
    for s in range(nstages):
        for l in (0, 1):
            nodes = sorted(i for (i, ll), st in stage.items()
                           if st == s and ll == l)
            for k in range(0, len(nodes), GROUP_MAX):
                groups.append((l, nodes[k:k + GROUP_MAX]))
    return groups


def _prep_weights(w_ih, w_hh):
    """Host-side weight prep -> [128, KC, 1024] fp16 stationary tiles.
    The 0.5 predecessor-mean is folded into W_hh."""

    def to_t(w):
        kdim = w.shape[1]
        wt = np.ascontiguousarray(w.T)            # [K, 1024]
        wt = wt.reshape(kdim // 128, 128, 1024)   # [kc, kin, 1024]
        return np.ascontiguousarray(
            wt.transpose(1, 0, 2).astype(np.float16))  # [128, kc, 1024]

    return to_t(w_ih), to_t(w_hh * 0.5)


def _build_program(pred):
    from contextlib import ExitStack
    from concourse import bacc, mybir, tile

    f32 = mybir.dt.float32
    f16 = mybir.dt.float16
    AF = mybir.ActivationFunctionType
    Alu = mybir.AluOpType

    groups = _build_schedule(pred)
    # layer-0 nodes in group order -> column order of the precomputed
    # x-gates stash (so per-group slices are contiguous)
    l0_nodes = [i for (l, nodes) in groups if l == 0 for i in nodes]
    l0_col = {i: k for k, i in enumerate(l0_nodes)}
    nl0 = len(l0_nodes)

    nc = bacc.Bacc("TRN2", target_bir_lowering=False, debug=False,
                   num_devices=NCORES)

    # dags gathered+transposed on host: [128, KC, nl0, 128] (group order)
    dags_t = nc.dram_tensor("dags_t", [128, KC, nl0, 128], f16,
                            kind="ExternalInput")
    h0_t = nc.dram_tensor("h0_t", [128, L, KC, 128], f16,
                          kind="ExternalInput")
    c0_t = nc.dram_tensor("c0_t", [128, L, KC, 128], f32,
                          kind="ExternalInput")
    w_dram = {}
    for l in range(L):
        w_dram[("x", l)] = nc.dram_tensor(f"wx{l}", [128, KC, 1024], f16,
                                          kind="ExternalInput")
        w_dram[("h", l)] = nc.dram_tensor(f"wh{l}", [128, KC, 1024], f16,
                                          kind="ExternalInput")
    # per-partition bias for activation bias APs: [128, L, 8] fp32
    bias_dram = nc.dram_tensor("bias_pp", [128, L, 8], f32,
                               kind="ExternalInput")
    out_t = nc.dram_tensor("out_t", [KC, 128, 128], f32, kind="ExternalOutput")

    with tile.TileContext(nc) as tc, ExitStack() as ctx:
        consts = ctx.enter_context(tc.tile_pool(name="consts", bufs=1))
        ps = ctx.enter_context(tc.tile_pool(name="ps", bufs=8, space="PSUM"))
        gp = ctx.enter_context(tc.tile_pool(name="gp", bufs=2))

        # --- input DMAs, latency-critical first ---
        bias_sb = consts.tile([128, L, 8], f32, tag="bias")
        wsb = {}
        for key in [("x", 0), ("h", 0), ("h", 1), ("x", 1)]:
            wsb[key] = consts.tile([128, KC, 1024], f16,
                                   tag=f"w{key[0]}{key[1]}",
                                   name=f"w{key[0]}{key[1]}")
        slot0_h = consts.tile([128, L, KC, 128], f16, tag="slot0h")
        slot0_c = consts.tile([128, L, KC, 128], f32, tag="slot0c")
        # xall split into two tiles so early groups don't depend on the
        # bulk transfer (tile-granular dependency tracking); the split must
        # fall on a group boundary
        bounds = [0]
        for (l, nodes) in groups:
            if l == 0:
                bounds.append(bounds[-1] + len(nodes))
        nfront = max((b for b in bounds if b <= 4), default=0)
        if nfront == 0:
            nfront = nl0
        nrest = nl0 - nfront
        xfront = consts.tile([128, KC, nfront, 128], f16, tag="xfront")
        xrest = None
        if nrest:
            xrest = consts.tile([128, KC, nrest, 128], f16, tag="xrest",
                                name="xrest")
        nc.sync.dma_start(out=bias_sb[:], in_=bias_dram[:])
        nc.sync.dma_start(out=xfront[:], in_=dags_t[:, :, :nfront])
        nc.sync.dma_start(out=wsb[("x", 0)][:, 0], in_=w_dram[("x", 0)][:, 0])
        nc.sync.dma_start(out=wsb[("h", 0)][:, 0], in_=w_dram[("h", 0)][:, 0])
        nc.sync.dma_start(out=slot0_h[:], in_=h0_t[:])
        nc.sync.dma_start(out=wsb[("h", 1)][:, 0], in_=w_dram[("h", 1)][:, 0])
        nc.sync.dma_start(out=wsb[("x", 1)][:, 0], in_=w_dram[("x", 1)][:, 0])
        nc.gpsimd.dma_start(out=wsb[("x", 0)][:, 1], in_=w_dram[("x", 0)][:, 1])
        nc.gpsimd.dma_start(out=wsb[("h", 0)][:, 1], in_=w_dram[("h", 0)][:, 1])
        nc.gpsimd.dma_start(out=slot0_c[:], in_=c0_t[:])
        if xrest is not None:
            nc.gpsimd.dma_start(out=xrest[:], in_=dags_t[:, :, nfront:])
        nc.gpsimd.dma_start(out=wsb[("h", 1)][:, 1], in_=w_dram[("h", 1)][:, 1])
        nc.gpsimd.dma_start(out=wsb[("x", 1)][:, 1], in_=w_dram[("x", 1)][:, 1])

        # --- PE warmup: dummy matmuls while input DMAs land ---
        warm = consts.tile([128, 256], f16, tag="warm")
        nc.vector.memset(warm[:], 0.0)
        wpt = ps.tile([128, GROUP_MAX * 128], f32, tag="gates",
                      name="gates")
        for _ in range(15):
            nc.tensor.matmul(out=wpt[:, :256], lhsT=warm[:, :128],
                             rhs=warm[:], start=(_ == 0), stop=(_ == 14))

        # per-group persistent state tiles; units are views [:, j]
        st_h = {}
        st_c = {}
        for g, (l, nodes) in enumerate(groups):
            u = len(nodes)
            gh = consts.tile([128, u, KC, 128], f16, tag=f"gh{g}",
                             name=f"gh{g}")
            gc = consts.tile([128, u, KC, 128], f32, tag=f"gc{g}",
                             name=f"gc{g}")
            for j, i in enumerate(nodes):
                st_h[(i, l)] = gh[:, j]
                st_c[(i, l)] = gc[:, j]
            groups[g] = (l, nodes, gh, gc)
        outh = consts.tile([128, KC, 128], f32, tag="outh")

        def h_ap(v, l):
            if v == 0:
                return slot0_h[:, l]
            return st_h[(v - 1, l)]

        def c_ap(v, l):
            if v == 0:
                return slot0_c[:, l]
            return st_c[(v - 1, l)]

        for (l, nodes, gh, gc) in groups:
            u = len(nodes)
            un = u * 128
            ubh = gp.tile([128, KC, u, 128], f16, tag="ubh")
            ubc = gp.tile([128, KC, u, 128], f32, tag="ubc")
            sifo = gp.tile([128, 4, u, 128], f16, tag="sifo")
            gt = gp.tile([128, KC, u, 128], f16, tag="gt")
            so = gp.tile([128, KC, u, 128], f16, tag="so")
            vw = gp.tile([128, KC, u, 128], f16, tag="vw")
            cf = gp.tile([128, KC, u, 128], f32, tag="cf")
            th = gp.tile([128, u, KC, 128], f16, tag="th")

            # 1. layer-1 input: copy of h_l0 (layer 0 reads xall direct)
            xq = None
            if l == 1:
                xq = gp.tile([128, KC, u, 128], f16, tag="xq", name="xq")
                for j, i in enumerate(nodes):
                    nc.vector.tensor_copy(out=xq[:, :, j, :],
                                          in_=st_h[(i, 0)])
            xcol = l0_col[nodes[0]] if l == 0 else 0

            # 2. predecessor state sums
            for j, i in enumerate(nodes):
                a, b_ = int(pred[i][0]), int(pred[i][1])
                if a == b_:
                    nc.vector.tensor_scalar_mul(ubh[:, :, j, :], h_ap(a, l),
                                                2.0)
                    nc.vector.tensor_scalar_mul(ubc[:, :, j, :], c_ap(a, l),
                                                2.0)
                else:
                    nc.vector.tensor_tensor(out=ubh[:, :, j, :],
                                            in0=h_ap(a, l), in1=h_ap(b_, l),
                                            op=Alu.add)
                    nc.vector.tensor_tensor(out=ubc[:, :, j, :],
                                            in0=c_ap(a, l), in1=c_ap(b_, l),
                                            op=Alu.add)

            def x_rhs(k):
                if l == 0:
                    if xcol + u <= nfront:
                        return xfront[:, k, xcol:xcol + u].rearrange(
                            "p u b -> p (u b)")
                    assert xcol >= nfront
                    return xrest[:, k, xcol - nfront:xcol - nfront + u] \
                        .rearrange("p u b -> p (u b)")
                return xq[:, k].rearrange("p u b -> p (u b)")

            # 3. per-chunk single-bank psum tiles (v1-style accumulation
            # groups with start=True: no cross-engine psum preload hazards).
            # Chunk order f,g,i,o so the c-path critical chain starts early.
            pm = {}
            morder = [2, 3, 4, 5, 0, 1, 6, 7]
            for m in morder:
                pm[m] = ps.tile([128, GROUP_MAX * 128], f32, tag="gates",
                                name="gates")
                for oi in range(2):
                    for k in range(KC):
                        nc.tensor.matmul(
                            out=pm[m][:, :un],
                            lhsT=wsb[("x" if oi == 0 else "h", l)]
                            [:, k, m * 128:(m + 1) * 128],
                            rhs=x_rhs(k) if oi == 0 else
                            ubh[:, k].rearrange("p u b -> p (u b)"),
                            start=(oi == 0 and k == 0),
                            stop=(oi == 1 and k == KC - 1))

            # 4. per-chunk activations with bias APs
            for m in morder:
                func = (AF.Tanh if m in (4, 5) else AF.Sigmoid)
                if m in (4, 5):
                    dst = gt[:, m - 4].rearrange("p u b -> p (u b)")
                elif m in (6, 7):
                    dst = so[:, m - 6].rearrange("p u b -> p (u b)")
                else:
                    dst = sifo[:, m].rearrange("p u b -> p (u b)")
                nc.scalar.activation(out=dst, in_=pm[m][:, :un], func=func,
                                     bias=bias_sb[:, l, m:m + 1])

            # 5. c_new = (ubc*0.5)*sigmoid(f) + sigmoid(i)*tanh(g)
            nc.vector.tensor_tensor(out=vw[:], in0=sifo[:, 0:2], in1=gt[:],
                                    op=Alu.mult)
            nc.vector.scalar_tensor_tensor(
                out=cf[:], in0=ubc[:], scalar=0.5, in1=sifo[:, 2:4],
                op0=Alu.mult, op1=Alu.mult)
            nc.vector.tensor_tensor(out=gc[:].rearrange("p u c b -> p c u b"),
                                    in0=cf[:], in1=vw[:], op=Alu.add)

            # 6. h = sigmoid(o) * tanh(c)
            nc.scalar.activation(out=th[:].rearrange("p u c b -> p (u c b)"),
                                 in_=gc[:].rearrange("p u c b -> p (u c b)"),
                                 func=AF.Tanh)
            nc.vector.tensor_tensor(out=gh[:].rearrange("p u c b -> p c u b"),
                                    in0=so[:],
                                    in1=th[:].rearrange("p u c b -> p c u b"),
                                    op=Alu.mult)
            if l == 1 and N - 1 in nodes:
                j = nodes.index(N - 1)
                nc.vector.tensor_tensor(out=outh[:], in0=so[:, :, j, :],
                                        in1=th[:, j], op=Alu.mult)

        # output: h of last node, top layer: [128, KC, 128] -> [KC, 128, 128]
        nc.sync.dma_start(out=out_t.ap().rearrange("k p b -> p k b"),
                          in_=outh[:])

    nc.compile()
    return nc, l0_nodes


def _get_program(pred):
    key = pred.tobytes()
    if key not in _CACHE:
        _CACHE[key] = _build_program(pred)
    return _CACHE[key]


def _prepare(dags, h0, c0, w_ih0, w_hh0, b_ih0, b_hh0,
             w_ih1, w_hh1, b_ih1, b_hh1, pred_idx):
    """Host-side prep: returns (nc, in_maps)."""
    dags = np.asarray(dags, dtype=np.float32)
    h0 = np.asarray(h0, dtype=np.float32)
    c0 = np.asarray(c0, dtype=np.float32)
    pred = np.asarray(pred_idx)

    nc, l0_nodes = _get_program(pred)

    wx0, wh0 = _prep_weights(np.asarray(w_ih0, np.float32),
                             np.asarray(w_hh0, np.float32))
    wx1, wh1 = _prep_weights(np.asarray(w_ih1, np.float32),
                             np.asarray(w_hh1, np.float32))
    bias = np.stack([
        np.asarray(b_ih0, np.float32) + np.asarray(b_hh0, np.float32),
        np.asarray(b_ih1, np.float32) + np.asarray(b_hh1, np.float32),
    ])  # [L, 1024]
    # per-partition bias [128, L, 8]: entry (p, l, m) = b_l[m*128 + p]
    bimg = np.ascontiguousarray(
        bias.reshape(L, 8, 128).transpose(2, 0, 1).astype(np.float32))

    in_maps = []
    for c in range(NCORES):
        bs = slice(c * BL, (c + 1) * BL)
        # dags [B, N, IN] -> cone l0 nodes (group order) ->
        # [128(p), KC, n, 128(b)] fp16
        sel = dags[bs][:, l0_nodes]                      # [BL, n, IN]
        dt_ = sel.transpose(2, 1, 0).reshape(KC, 128, len(l0_nodes), BL)
        dt_ = np.ascontiguousarray(
            dt_.transpose(1, 0, 2, 3).astype(np.float16))
        # h0/c0 [L, B, H] -> [128(p), L, kc, b]
        hh = h0[:, bs, :].transpose(2, 0, 1).reshape(KC, 128, L, BL)
        cc = c0[:, bs, :].transpose(2, 0, 1).reshape(KC, 128, L, BL)
        h0t = np.ascontiguousarray(
            hh.transpose(1, 2, 0, 3).astype(np.float16))  # [128, L, kc, b]
        c0t = np.ascontiguousarray(
            cc.transpose(1, 2, 0, 3).astype(np.float32))
        in_maps.append({
            "dags_t": dt_, "h0_t": h0t, "c0_t": c0t,
            "wx0": wx0, "wh0": wh0, "wx1": wx1, "wh1": wh1,
            "bias_pp": bimg,
        })
    return nc, in_maps


def _assemble(res):
    out = np.empty((B, H), np.float32)
    for c in range(NCORES):
        ot = res.results[c]["out_t"]  # [KC, 128, 128] = [kc, p, b]
        out[c * BL:(c + 1) * BL] = ot.reshape(H, BL).T
    return out


def kernel(**inputs):
    from concourse.bass_utils import run_bass_kernel_spmd

    nc, in_maps = _prepare(**inputs)
    res = run_bass_kernel_spmd(nc, in_maps, list(range(NCORES)))
    return _assemble(res)
